# revision 1
# baseline (speedup 1.0000x reference)
"""Trainium2 Bass kernel for EnhancedAttention (B=2, S=2048, DM=1024, H=16, D=64).

Strategy: 8 NeuronCores = data-parallel over batch (2) x tensor-parallel over
heads (4 heads/core). Each core runs a fused QKV-projection + attention +
output-projection program (fp32 storage, float32r matmuls); the host sums the
4 output-projection partials per batch element and applies the biases.

kernel(**inputs) takes the full unsharded inputs and returns the full output.
"""

import os
import sys

for _p in ("/opt/trn_rl_repo", "/root/.axon_site/_ro/trn_rl_repo"):
    if os.path.isdir(_p) and _p not in sys.path:
        sys.path.append(_p)

import numpy as np
from contextlib import ExitStack

import concourse.bass as bass
import concourse.mybir as mybir
import concourse.tile as tile
from concourse import bacc
from concourse.bass import ts, ds

F32 = mybir.dt.float32
F32R = mybir.dt.float32r
EXP = mybir.ActivationFunctionType.Exp
ADD = mybir.AluOpType.add
MULT = mybir.AluOpType.mult

S = 2048
DM = 1024
HD = 64


def build_program(use_f32r=True, repeat=1):
    MMDT = F32R if use_f32r else F32
    nc = bacc.Bacc("TRN2", debug=False)
    xt = nc.dram_tensor("xt", [DM, S], MMDT, kind="ExternalInput").ap()
    wq = nc.dram_tensor("wq", [DM, 256], MMDT, kind="ExternalInput").ap()
    wk = nc.dram_tensor("wk", [DM, 256], MMDT, kind="ExternalInput").ap()
    wv = nc.dram_tensor("wv", [DM, 256], MMDT, kind="ExternalInput").ap()
    bqk = nc.dram_tensor("bqk", [128, 4], F32, kind="ExternalInput").ap()
    post = nc.dram_tensor("post", [128, S], F32, kind="ExternalInput").ap()
    wo = nc.dram_tensor("wo", [256, DM], MMDT, kind="ExternalInput").ap()
    out = nc.dram_tensor("out", [S, DM], F32, kind="ExternalOutput").ap()

    with tile.TileContext(nc) as tc, ExitStack() as ctx:
        p_xt = ctx.enter_context(tc.tile_pool(name="xt", bufs=32))
        p_w = ctx.enter_context(tc.tile_pool(name="w", bufs=24))
        p_sing = ctx.enter_context(tc.tile_pool(name="sing", bufs=1))
        p_wo = ctx.enter_context(tc.tile_pool(name="wo", bufs=2))
        p_qt = ctx.enter_context(tc.tile_pool(name="qt", bufs=8))
        p_kt = ctx.enter_context(tc.tile_pool(name="kt", bufs=8))
        p_va = ctx.enter_context(tc.tile_pool(name="va", bufs=16))
        p_es = ctx.enter_context(tc.tile_pool(name="es", bufs=6))
        p_ot = ctx.enter_context(tc.tile_pool(name="ot", bufs=2))
        p_rc = ctx.enter_context(tc.tile_pool(name="rc", bufs=2))
        p_rb = ctx.enter_context(tc.tile_pool(name="rb", bufs=2))
        p_fo = ctx.enter_context(tc.tile_pool(name="fo", bufs=3))
        p_dr = ctx.enter_context(tc.tile_pool(name="dr", bufs=2, space="DRAM"))
        ps_sc = ctx.enter_context(tc.tile_pool(name="pssc", bufs=2, space="PSUM"))
        ps_acc = ctx.enter_context(tc.tile_pool(name="psacc", bufs=4, space="PSUM"))

        for _rep in range(repeat):
            # ---- input DMAs, critical-path order:
            # bqk+pos gate the first projection drains; wk/wq gate KQ0;
            # xt qb-major gates each proj piece; wv/wo needed later.
            bqk_sb = p_sing.tile([128, 4], F32, tag="bqk", name=f"bqk{_rep}")
            nc.gpsimd.dma_start(out=bqk_sb, in_=bqk)
            pos_sb = p_sing.tile([128, S], F32, tag="post", name=f"pos{_rep}")
            nc.gpsimd.dma_start(out=pos_sb, in_=post)
            w_sb = {}

            def dma_w(name, dram):
                tiles = []
                for t in range(8):
                    w_t = p_w.tile(
                        [128, 256], MMDT, tag="w", name=f"w{name}{_rep}_{t}"
                    )
                    nc.sync.dma_start(out=w_t, in_=dram[ts(t, 128), :])
                    tiles.append(w_t)
                w_sb[name] = tiles

            xts = [[None] * 8 for _ in range(4)]

            def dma_xt(qb, eng=None):
                e = eng if eng is not None else nc.sync
                for t in range(8):
                    x_t = p_xt.tile(
                        [128, 512], MMDT, tag="xt", name=f"x_t{_rep}_{qb}_{t}"
                    )
                    e.dma_start(
                        out=x_t, in_=xt[ts(t, 128), ts(qb, 512)]
                    )
                    xts[qb][t] = x_t

            dma_w("k", wk)
            dma_w("q", wq)
            dma_xt(0)
            tiles = []
            for t in range(8):
                w_t = p_w.tile([128, 256], MMDT, tag="w", name=f"wv{_rep}_{t}")
                nc.sync.dma_start(out=w_t, in_=wv[ts(t, 128), :])
                tiles.append(w_t)
            w_sb["v"] = tiles
            dma_xt(1)
            dma_xt(2)
            dma_xt(3)
            wo_sb = []
            for t in range(2):
                wo_t = p_wo.tile([128, DM], MMDT, tag="wo", name=f"wo{_rep}_{t}")
                nc.sync.dma_start(out=wo_t, in_=wo[ts(t, 128), :])
                wo_sb.append(wo_t)

            QT = [
                [
                    p_qt.tile([128, 512], MMDT, tag="qt", name=f"QT{_rep}_{g}_{qb}")
                    for qb in range(4)
                ]
                for g in range(2)
            ]
            KT = [
                [
                    p_kt.tile([128, 512], MMDT, tag="kt", name=f"KT{_rep}_{g}_{qb}")
                    for qb in range(4)
                ]
                for g in range(2)
            ]
            OT = [
                p_ot.tile([128, S], MMDT, tag="ot", name=f"OT{_rep}_{g}")
                for g in range(2)
            ]

            # ---- K/Q projections (transposed layout)
            def proj_kq(g):
                for qb in range(4):
                    psk = ps_sc.tile([128, 512], F32, tag="pssc")
                    for t in range(8):
                        nc.tensor.matmul(
                            psk,
                            w_sb["k"][t][:, ds(g * 128, 128)],
                            xts[qb][t],
                            start=(t == 0),
                            stop=(t == 7),
                        )
                    nc.vector.scalar_tensor_tensor(
                        out=KT[g][:, ts(qb, 512)],
                        in0=psk,
                        scalar=bqk_sb[:, ds(2 + g, 1)],
                        in1=pos_sb[:, ts(qb, 512)],
                        op0=ADD,
                        op1=ADD,
                    )
                    psq = ps_sc.tile([128, 512], F32, tag="pssc")
                    for t in range(8):
                        nc.tensor.matmul(
                            psq,
                            w_sb["q"][t][:, ds(g * 128, 128)],
                            xts[qb][t],
                            start=(t == 0),
                            stop=(t == 7),
                        )
                    nc.vector.tensor_scalar_add(
                        QT[g][:, ts(qb, 512)], psq, bqk_sb[:, ds(g, 1)]
                    )

            # ---- V projection (natural layout) + ones column per head
            VA = []

            def proj_v():
                for st in range(16):
                    va = p_va.tile(
                        [128, 260], MMDT, tag="va", name=f"va{_rep}_{st}"
                    )
                    psv = ps_sc.tile([128, 256], F32, tag="pssc")
                    for t in range(8):
                        nc.tensor.matmul(
                            psv,
                            xts[st // 4][t][:, ds((st % 4) * 128, 128)],
                            w_sb["v"][t],
                            start=(t == 0),
                            stop=(t == 7),
                        )
                    va_r = va.rearrange("p (h d) -> p h d", h=4)
                    psv_r = psv.rearrange("p (h d) -> p h d", h=4)
                    nc.vector.tensor_copy(va_r[:, :, 0:64], psv_r)
                    # ones column per head: out = in*0 + 1
                    nc.vector.tensor_scalar(
                        out=va_r[:, :, 64:65],
                        in0=psv_r[:, :, 0:1],
                        scalar1=0.0,
                        scalar2=1.0,
                        op0=MULT,
                        op1=ADD,
                    )
                    VA.append(va)

            # ---- emission framework: per-(kt,qq) bundles ----------------
            # Each bundle: 2 score MMs (both heads, disjoint row groups) into
            # one [128,1024] psum tile + 1 exp. PV work trails via a queue
            # with a 2-bundle lag; "extras" (projection pieces, final-proj
            # tiles) are sprinkled one per bundle to fill PE slack.
            es_store = {}      # (qh, g, qq, kt) -> es tile
            otps_store = {}    # (qh, g) -> otps[h][qq]
            pv_queue = []      # deferred pv closures

            def make_otps(qh, g):
                otps_store[(qh, g)] = [
                    [
                        ps_acc.tile(
                            [65, 512],
                            F32,
                            tag="psacc",
                            name=f"otp{_rep}_{qh}{g}{h}{qq}",
                        )
                        for qq in range(2)
                    ]
                    for h in range(2)
                ]

            def normalize(qh, g, h, qq):
                otp = otps_store[(qh, g)][h][qq]
                rc = p_rc.tile([1, 512], F32, tag="rc")
                nc.vector.reciprocal(rc, otp[ds(64, 1), :])
                rd = p_dr.tile([1, 512], F32, tag="rd")
                nc.gpsimd.dma_start(out=rd, in_=rc)
                rb = p_rb.tile([64, 512], F32, tag="rb")
                rd_b = bass.AP(
                    tensor=rd.tensor,
                    offset=rd.offset,
                    ap=[[0, 64]] + list(rd.ap[1:]),
                )
                nc.gpsimd.dma_start(out=rb, in_=rd_b)
                nc.vector.tensor_mul(
                    OT[g][ds(h * 64, 64), ds(qh * 1024 + qq * 512, 512)],
                    otp[ds(0, 64), :],
                    rb,
                )

            def pv_work(qh, g, kt, qq):
                def f():
                    otps = otps_store[(qh, g)]
                    es = es_store[(qh, g, qq, kt)]
                    for h in range(2):
                        nc.tensor.matmul(
                            otps[h][qq],
                            VA[kt][:, ds((g * 2 + h) * 65, 65)],
                            es[:, ts(h, 512)],
                            start=(kt == 0),
                            stop=(kt == 15),
                        )
                    if kt == 15:
                        for h in range(2):
                            normalize(qh, g, h, qq)
                return f

            def bundle(qh, g, kt, qq):
                sc = ps_sc.tile(
                    [128, 1024], F32, tag="pssc",
                    name=f"sc{_rep}_{qh}{g}{kt}{qq}",
                )
                for h in range(2):
                    nc.tensor.matmul(
                        sc[:, ts(h, 512)],
                        KT[g][kt // 4][ds(h * 64, 64), ts(kt % 4, 128)],
                        QT[g][qh * 2 + qq][ds(h * 64, 64), :],
                        start=True,
                        stop=True,
                    )
                es = p_es.tile([128, 1024], MMDT, tag="es")
                nc.scalar.activation(es, sc, EXP, scale=0.125)
                es_store[(qh, g, qq, kt)] = es
                pv_queue.append(pv_work(qh, g, kt, qq))

            # ---- extra work pieces (one per bundle slot) -----------------
            def kq_piece(g, qb, which, acc=False):
                def f():
                    pool, tg = (ps_acc, "psacc") if acc else (ps_sc, "pssc")
                    if which == 0:
                        psk = pool.tile([128, 512], F32, tag=tg)
                        for t in range(8):
                            nc.tensor.matmul(
                                psk,
                                w_sb["k"][t][:, ds(g * 128, 128)],
                                xts[qb][t],
                                start=(t == 0),
                                stop=(t == 7),
                            )
                        nc.vector.scalar_tensor_tensor(
                            out=KT[g][qb],
                            in0=psk,
                            scalar=bqk_sb[:, ds(2 + g, 1)],
                            in1=pos_sb[:, ts(qb, 512)],
                            op0=ADD,
                            op1=ADD,
                        )
                    else:
                        psq = pool.tile([128, 512], F32, tag=tg)
                        for t in range(8):
                            nc.tensor.matmul(
                                psq,
                                w_sb["q"][t][:, ds(g * 128, 128)],
                                xts[qb][t],
                                start=(t == 0),
                                stop=(t == 7),
                            )
                        nc.vector.tensor_scalar_add(
                            QT[g][qb], psq, bqk_sb[:, ds(g, 1)]
                        )
                return f

            def v_piece(st, acc=False):
                def f():
                    va = p_va.tile(
                        [128, 260], MMDT, tag="va", name=f"va{_rep}_{st}"
                    )
                    pool, tg = (ps_acc, "psacc") if acc else (ps_sc, "pssc")
                    psv = pool.tile([128, 256], F32, tag=tg)
                    for t in range(8):
                        nc.tensor.matmul(
                            psv,
                            xts[st // 4][t][:, ds((st % 4) * 128, 128)],
                            w_sb["v"][t],
                            start=(t == 0),
                            stop=(t == 7),
                        )
                    va_r = va.rearrange("p (h d) -> p h d", h=4)
                    psv_r = psv.rearrange("p (h d) -> p h d", h=4)
                    nc.vector.tensor_copy(va_r[:, :, 0:64], psv_r)
                    nc.vector.tensor_scalar(
                        out=va_r[:, :, 64:65],
                        in0=psv_r[:, :, 0:1],
                        scalar1=0.0,
                        scalar2=1.0,
                        op0=MULT,
                        op1=ADD,
                    )
                    VA.append(va)
                return f

            def fin_piece(qt_i, small=False):
                def f():
                    if small:
                        # two [128,512] pieces through ps_acc (fits the
                        # [65,512]-sized slots; avoids stalling the scores
                        # pool while exps are still streaming)
                        for nb in range(2):
                            fo = ps_acc.tile([128, 512], F32, tag="psacc")
                            for hdt in range(2):
                                nc.tensor.matmul(
                                    fo,
                                    OT[hdt][:, ts(qt_i, 128)],
                                    wo_sb[hdt][:, ts(nb, 512)],
                                    start=(hdt == 0),
                                    stop=(hdt == 1),
                                )
                            fs = p_fo.tile([128, 512], F32, tag="fo")
                            nc.vector.tensor_copy(fs, fo)
                            nc.sync.dma_start(
                                out=out[ts(qt_i, 128), ts(nb, 512)], in_=fs
                            )
                        return
                    fo = ps_sc.tile([128, 1024], F32, tag="pssc")
                    for nb in range(2):
                        for hdt in range(2):
                            nc.tensor.matmul(
                                fo[:, ts(nb, 512)],
                                OT[hdt][:, ts(qt_i, 128)],
                                wo_sb[hdt][:, ts(nb, 512)],
                                start=(hdt == 0),
                                stop=(hdt == 1),
                            )
                    for nb in range(2):
                        fs = p_fo.tile([128, 512], F32, tag="fo")
                        nc.vector.tensor_copy(fs, fo[:, ts(nb, 512)])
                        nc.sync.dma_start(
                            out=out[ts(qt_i, 128), ts(nb, 512)], in_=fs
                        )
                return f

            # ---- schedule ------------------------------------------------
            PV_LAG = 4

            def run_block(qh, g, order, extras):
                make_otps(qh, g)
                ei = 0
                for i, (kt, qq) in enumerate(order):
                    bundle(qh, g, kt, qq)
                    while len(pv_queue) > PV_LAG:
                        pv_queue.pop(0)()
                    if ei < len(extras):
                        if extras[ei] is not None:
                            extras[ei]()
                        ei += 1
                while ei < len(extras):
                    if extras[ei] is not None:
                        extras[ei]()
                    ei += 1

            def drain_pv():
                while pv_queue:
                    pv_queue.pop(0)()

            # lead: K/Q for pair 0, qb0 (gates the first bundles)
            kq_piece(0, 0, 0)()
            kq_piece(0, 0, 1)()

            # A = (qh0, g0): bundles ordered by DMA/projection availability;
            # extras deliver V (paced for the PV lag) and the rest of KQ0.
            orderA = (
                [(kt, 0) for kt in range(4)]
                + [(kt, 1) for kt in range(4)]
                + [(kt, 0) for kt in range(4, 8)]
                + [(kt, 1) for kt in range(4, 8)]
                + [(kt, qq) for kt in range(8, 12) for qq in (0, 1)]
                + [(kt, 0) for kt in range(12, 16)]
                + [(kt, 1) for kt in range(12, 16)]
            )
            extrasA = [
                v_piece(0, acc=True), kq_piece(0, 1, 1, acc=True),
                v_piece(1, acc=True), v_piece(2, acc=True),
                kq_piece(0, 1, 0, acc=True), v_piece(3, acc=True),
                v_piece(4), v_piece(5),
                kq_piece(0, 2, 0), v_piece(6), v_piece(7), kq_piece(0, 3, 0),
                v_piece(8), v_piece(9), v_piece(10), v_piece(11),
                kq_piece(0, 2, 1), kq_piece(0, 3, 1), v_piece(12),
                v_piece(13), v_piece(14), v_piece(15),
            ]
            run_block(0, 0, orderA, extrasA)

            # B = (qh1, g0): interleave KQ1
            orderB = [(kt, qq) for kt in range(16) for qq in (0, 1)]
            extrasB = [
                kq_piece(1, qb, w, acc=(qb * 2 + w < 4))
                for qb in range(4)
                for w in range(2)
            ]
            run_block(1, 0, orderB, extrasB)

            # C = (qh0, g1)
            run_block(0, 1, orderB, [])

            # D = (qh1, g1): qq-major so qq0's accumulators retire early;
            # final(qh0) tiles then reuse the freed ps_acc slots.
            orderD = [(kt, 0) for kt in range(16)] + [(kt, 1) for kt in range(16)]
            extrasD = [None] * 32
            for _i, _q8 in enumerate(range(8)):
                extrasD[6 + 2 * _i] = fin_piece(_q8, small=True)
            run_block(1, 1, orderD, extrasD)
            drain_pv()
            for q8 in range(8, 16):
                fin_piece(q8)()


    nc.compile()
    return nc


# ---------------- host-side helpers ----------------


def rel_pos_enc(seq_len, dim):
    positions = np.arange(seq_len, dtype=np.float32)[:, None]
    div_term = np.exp(
        np.arange(0, dim, 2, dtype=np.float32) * (-(np.log(10000.0) / dim))
    )
    pe = np.zeros((seq_len, dim), dtype=np.float32)
    pe[:, 0::2] = np.sin(positions * div_term)
    pe[:, 1::2] = np.cos(positions * div_term)
    return pe


def core_inputs(x, W_qkv, b_qkv, core):
    b = core // 4
    h0 = (core % 4) * 4
    cols = slice(h0 * 64, (h0 + 4) * 64)
    xt = np.ascontiguousarray(x[b].T)
    wq = np.ascontiguousarray(W_qkv[:, 0:1024][:, cols])
    wk = np.ascontiguousarray(W_qkv[:, 1024:2048][:, cols])
    wv = np.ascontiguousarray(W_qkv[:, 2048:3072][:, cols])
    bq = b_qkv[0:1024][cols]
    bk = b_qkv[1024:2048][cols]
    bqk = np.stack(
        [bq[0:128], bq[128:256], bk[0:128], bk[128:256]], axis=1
    ).astype(np.float32)
    pos = rel_pos_enc(S, HD)  # [S, 64]
    post = np.ascontiguousarray(
        np.concatenate([pos.T, pos.T], axis=0).astype(np.float32)
    )  # [128, S]
    return {
        "xt": xt,
        "wq": wq,
        "wk": wk,
        "wv": wv,
        "bqk": np.ascontiguousarray(bqk),
        "post": post,
    }


def core_inputs_out(W_out, core):
    h0 = (core % 4) * 4
    rows = slice(h0 * 64, (h0 + 4) * 64)
    return {"wo": np.ascontiguousarray(W_out[rows, :])}


def all_core_inputs(x, W_qkv, b_qkv, W_out):
    ins = []
    for c in range(8):
        m = core_inputs(x, W_qkv, b_qkv, c)
        m.update(core_inputs_out(W_out, c))
        ins.append(m)
    return ins


def combine_outputs(partials, b_qkv, W_out, b_out):
    extra = b_qkv[2048:3072] @ W_out + b_out  # [DM]
    outs = []
    for b in range(2):
        acc = partials[b * 4].astype(np.float64)
        for c in range(b * 4 + 1, b * 4 + 4):
            acc = acc + partials[c]
        outs.append((acc + extra).astype(np.float32))
    return np.stack(outs, axis=0)  # [2, S, DM]


_CACHE = {}


def _get_program():
    if "nc" not in _CACHE:
        _CACHE["nc"] = build_program(use_f32r=True)
    return _CACHE["nc"]


def kernel(x, W_qkv, b_qkv, W_out, b_out):
    x = np.ascontiguousarray(np.asarray(x, dtype=np.float32))
    W_qkv = np.ascontiguousarray(np.asarray(W_qkv, dtype=np.float32))
    b_qkv = np.asarray(b_qkv, dtype=np.float32)
    W_out = np.ascontiguousarray(np.asarray(W_out, dtype=np.float32))
    b_out = np.asarray(b_out, dtype=np.float32)

    from concourse import bass_utils

    nc = _get_program()
    in_maps = all_core_inputs(x, W_qkv, b_qkv, W_out)
    res = bass_utils.run_bass_kernel_spmd(nc, in_maps, core_ids=list(range(8)))
    partials = [res.results[c]["out"] for c in range(8)]
    return combine_outputs(partials, b_qkv, W_out, b_out)



# revision 17
# speedup vs baseline: 1.5275x; 1.5275x over previous
"""Trainium2 Bass kernel for EnhancedAttention (B=2, S=2048, DM=1024, H=16, D=64).

Strategy: 8 NeuronCores = data-parallel over batch (2) x tensor-parallel over
heads (4 heads/core). Each core runs a fused QKV-projection + attention +
output-projection program; the host sums the 4 output-projection partials per
batch element and applies the biases.

Schedule (v2): four attention blocks (qh, g) in qh-major order, each block
qq-major (all 16 key tiles for query half 0, then half 1). Projection and
final-projection pieces are interleaved as per-bundle "extras" so the PE never
starves while the activation engine streams the exps. Softmax denominators
come out of the PV matmul itself: each head's V tile is [v(64) | ones(64)], so
PSUM rows 64-127 hold sum(exp) already broadcast across 64 partitions and the
normalize is a DVE reciprocal+multiply with no DMA round-trip. QT/KT/es/VA are
bf16 (same PE rate, half the SBUF); accumulation stays fp32.

kernel(**inputs) takes the full unsharded inputs and returns the full output.
"""

import os
import sys

for _p in ("/opt/trn_rl_repo", "/root/.axon_site/_ro/trn_rl_repo"):
    if os.path.isdir(_p) and _p not in sys.path:
        sys.path.append(_p)

import numpy as np
from contextlib import ExitStack

import concourse.bass as bass
import concourse.mybir as mybir
import concourse.tile as tile
from concourse import bacc
from concourse.bass import ts, ds

F32 = mybir.dt.float32
F32R = mybir.dt.float32r
BF16 = mybir.dt.bfloat16
EXP = mybir.ActivationFunctionType.Exp
COPY = mybir.ActivationFunctionType.Copy
ADD = mybir.AluOpType.add
MULT = mybir.AluOpType.mult

S = 2048
DM = 1024
HD = 64
PV_LAG = 5


class _Rep:
    """Per-repetition tile state."""

    __slots__ = ("xts", "w_sb", "bqk_sb", "pos_sb", "wo_sb",
                 "QT", "KT", "OT", "VA", "pv_queue")

    def __init__(self):
        self.xts = [[None] * 8 for _ in range(4)]
        self.w_sb = {}
        self.QT = [[None] * 4 for _ in range(2)]   # [g][qb]
        self.KT = [[None] * 4 for _ in range(2)]   # [g][qb]
        self.OT = [None, None]                     # [g]
        self.VA = [None] * 16
        self.pv_queue = []


def build_program(use_f32r=True, repeat=1):
    MMDT = F32R if use_f32r else F32
    nc = bacc.Bacc("TRN2", debug=False)
    xt = nc.dram_tensor("xt", [DM, S], MMDT, kind="ExternalInput").ap()
    wq = nc.dram_tensor("wq", [DM, 256], MMDT, kind="ExternalInput").ap()
    wk = nc.dram_tensor("wk", [DM, 256], MMDT, kind="ExternalInput").ap()
    wv = nc.dram_tensor("wv", [DM, 256], MMDT, kind="ExternalInput").ap()
    bqk = nc.dram_tensor("bqk", [128, 4], F32, kind="ExternalInput").ap()
    post = nc.dram_tensor("post", [128, S], F32, kind="ExternalInput").ap()
    wo = nc.dram_tensor("wo", [256, DM], MMDT, kind="ExternalInput").ap()
    out = nc.dram_tensor("out", [S, DM], F32, kind="ExternalOutput").ap()

    with tile.TileContext(nc) as tc, ExitStack() as ctx:
        p_xt = ctx.enter_context(tc.tile_pool(name="xt", bufs=32))
        p_w = ctx.enter_context(tc.tile_pool(name="w", bufs=24))
        p_sing = ctx.enter_context(tc.tile_pool(name="sing", bufs=2))
        p_wo = ctx.enter_context(tc.tile_pool(name="wo", bufs=4))
        p_qt = ctx.enter_context(tc.tile_pool(name="qt", bufs=8))
        p_kt = ctx.enter_context(tc.tile_pool(name="kt", bufs=8))
        p_va = ctx.enter_context(tc.tile_pool(name="va", bufs=16))
        p_es = ctx.enter_context(tc.tile_pool(name="es", bufs=8))
        p_ot = ctx.enter_context(tc.tile_pool(name="ot", bufs=2))
        p_rc = ctx.enter_context(tc.tile_pool(name="rc", bufs=2))
        p_fo = ctx.enter_context(tc.tile_pool(name="fo", bufs=3))
        ps_b = ctx.enter_context(tc.tile_pool(name="psb", bufs=2, space="PSUM"))
        ps_o = ctx.enter_context(tc.tile_pool(name="pso", bufs=2, space="PSUM"))

        reps = [_Rep() for _ in range(repeat)]

        # ---- DMA emission ------------------------------------------------
        def dmas(r):
            st = reps[r]
            st.bqk_sb = p_sing.tile([128, 4], F32, tag="bqk", name=f"bqk{r}")
            nc.gpsimd.dma_start(out=st.bqk_sb, in_=bqk)
            st.pos_sb = p_sing.tile([128, S], F32, tag="post", name=f"pos{r}")
            nc.gpsimd.dma_start(out=st.pos_sb, in_=post)

            def dma_w(nm, dram):
                tiles = []
                for t in range(8):
                    w_t = p_w.tile([128, 256], MMDT, tag="w",
                                   name=f"w{nm}{r}_{t}")
                    nc.sync.dma_start(out=w_t, in_=dram[ts(t, 128), :])
                    tiles.append(w_t)
                st.w_sb[nm] = tiles

            def dma_xt(qb):
                for t in range(8):
                    x_t = p_xt.tile([128, 512], MMDT, tag="xt",
                                    name=f"x_t{r}_{qb}_{t}")
                    nc.sync.dma_start(out=x_t, in_=xt[ts(t, 128), ts(qb, 512)])
                    st.xts[qb][t] = x_t

            dma_w("k", wk)
            dma_w("q", wq)
            dma_xt(0)
            dma_w("v", wv)
            dma_xt(1)
            dma_xt(2)
            dma_xt(3)
            st.wo_sb = []
            for t in range(2):
                wo_t = p_wo.tile([128, DM], MMDT, tag="wo", name=f"wo{r}_{t}")
                nc.sync.dma_start(out=wo_t, in_=wo[ts(t, 128), :])
                st.wo_sb.append(wo_t)

        # ---- projection pieces ------------------------------------------
        def kq_piece(r, g, qb, which):
            def f():
                st = reps[r]
                if which == 0:
                    psk = ps_b.tile([128, 512], F32, tag="psb",
                                    name=f"psk{r}_{g}{qb}")
                    for t in range(8):
                        nc.tensor.matmul(
                            psk, st.w_sb["k"][t][:, ds(g * 128, 128)],
                            st.xts[qb][t], start=(t == 0), stop=(t == 7))
                    kt_t = p_kt.tile([128, 512], BF16, tag="kt",
                                     name=f"KT{r}_{g}_{qb}")
                    nc.vector.scalar_tensor_tensor(
                        out=kt_t, in0=psk, scalar=st.bqk_sb[:, ds(2 + g, 1)],
                        in1=st.pos_sb[:, ts(qb, 512)], op0=ADD, op1=ADD)
                    st.KT[g][qb] = kt_t
                else:
                    psq = ps_b.tile([128, 512], F32, tag="psb",
                                    name=f"psq{r}_{g}{qb}")
                    for t in range(8):
                        nc.tensor.matmul(
                            psq, st.w_sb["q"][t][:, ds(g * 128, 128)],
                            st.xts[qb][t], start=(t == 0), stop=(t == 7))
                    qt_t = p_qt.tile([128, 512], BF16, tag="qt",
                                     name=f"QT{r}_{g}_{qb}")
                    nc.vector.tensor_scalar_add(
                        qt_t, psq, st.bqk_sb[:, ds(g, 1)])
                    st.QT[g][qb] = qt_t
            return f

        def v_piece(r, stt):
            def f():
                st = reps[r]
                va = p_va.tile([128, 512], BF16, tag="va", name=f"va{r}_{stt}")
                va_r = va.rearrange("p (h c) -> p h c", h=4)
                # ones half-block per head, emitted first so the DVE write
                # overlaps the projection matmuls
                nc.vector.memset(va_r[:, :, 64:128], 1.0)
                psv = ps_b.tile([128, 256], F32, tag="psb",
                                name=f"psv{r}_{stt}")
                for t in range(8):
                    nc.tensor.matmul(
                        psv, st.xts[stt // 4][t][:, ds((stt % 4) * 128, 128)],
                        st.w_sb["v"][t], start=(t == 0), stop=(t == 7))
                psv_r = psv.rearrange("p (h d) -> p h d", h=4)
                nc.vector.tensor_copy(va_r[:, :, 0:64], psv_r)
                st.VA[stt] = va
            return f

        def fin_piece(r, qt_i, pool=None):
            def f():
                st = reps[r]
                pl, tg = (ps_o, "otp") if pool == "o" else (ps_b, "psb")
                fo = pl.tile([128, 1024], F32, tag=tg,
                             name=f"fin{r}_{qt_i}")
                for nb in range(2):
                    for hdt in range(2):
                        nc.tensor.matmul(
                            fo[:, ts(nb, 512)],
                            st.OT[hdt][:, ts(qt_i, 128)],
                            st.wo_sb[hdt][:, ts(nb, 512)],
                            start=(hdt == 0), stop=(hdt == 1))
                for nb in range(2):
                    fs = p_fo.tile([128, 512], F32, tag="fo",
                                   name=f"fs{r}_{qt_i}_{nb}")
                    if nb == 0:
                        nc.scalar.activation(fs, fo[:, ts(nb, 512)], COPY)
                    else:
                        nc.vector.tensor_copy(fs, fo[:, ts(nb, 512)])
                    nc.sync.dma_start(
                        out=out[ts(qt_i, 128), ts(nb, 512)], in_=fs)
            return f

        # ---- attention block machinery -----------------------------------
        def normalize(r, otp, qh, g, h, qq):
            st = reps[r]
            rc = p_rc.tile([64, 512], F32, tag="rc", name=f"rc{r}_{qh}{g}{h}{qq}")
            nc.vector.reciprocal(rc, otp[ds(64, 64), ts(h, 512)])
            nc.vector.tensor_mul(
                st.OT[g][ds(h * 64, 64), ds(qh * 1024 + qq * 512, 512)],
                otp[ds(0, 64), ts(h, 512)], rc)

        def run_block(r, qh, g, extras, lag=PV_LAG):
            st = reps[r]
            if st.OT[g] is None:
                st.OT[g] = p_ot.tile([128, S], MMDT, tag="ot",
                                     name=f"OT{r}_{g}")
            es_store = {}
            otp_store = {}

            def pv_work(kt, qq):
                def f():
                    if kt == 0:
                        otp_store[qq] = ps_o.tile(
                            [128, 1024], F32, tag="otp",
                            name=f"otp{r}_{qh}{g}{qq}")
                    otp = otp_store[qq]
                    es = es_store.pop((qq, kt))
                    for h in range(2):
                        nc.tensor.matmul(
                            otp[:, ts(h, 512)],
                            st.VA[kt][:, ds((g * 2 + h) * 128, 128)],
                            es[:, ts(h, 512)],
                            start=(kt == 0), stop=(kt == 15))
                    if kt == 15:
                        for h in range(2):
                            normalize(r, otp, qh, g, h, qq)
                return f

            order = [(kt, 0) for kt in range(16)] + [(kt, 1) for kt in range(16)]
            for i, (kt, qq) in enumerate(order):
                while len(st.pv_queue) > lag:
                    st.pv_queue.pop(0)()
                if i < len(extras) and extras[i] is not None:
                    extras[i]()
                sc = ps_b.tile([128, 1024], F32, tag="psb",
                               name=f"sc{r}_{qh}{g}{kt}{qq}")
                for h in range(2):
                    nc.tensor.matmul(
                        sc[:, ts(h, 512)],
                        st.KT[g][kt // 4][ds(h * 64, 64), ts(kt % 4, 128)],
                        st.QT[g][qh * 2 + qq][ds(h * 64, 64), :],
                        start=True, stop=True)
                es = p_es.tile([128, 1024], BF16, tag="es",
                               name=f"es{r}_{qh}{g}{kt}{qq}")
                nc.scalar.activation(es, sc, EXP, scale=0.125)
                es_store[(qq, kt)] = es
                st.pv_queue.append(pv_work(kt, qq))
            for i in range(len(order), len(extras)):
                if extras[i] is not None:
                    extras[i]()

        def drain_pv(r):
            st = reps[r]
            while st.pv_queue:
                st.pv_queue.pop(0)()

        # ---- schedule ----------------------------------------------------
        dmas(0)
        kq_piece(0, 0, 0, 0)()
        kq_piece(0, 0, 0, 1)()

        for r in range(repeat):
            kq = lambda g, qb, w: kq_piece(r, g, qb, w)
            v = lambda stt: v_piece(r, stt)

            extrasA = [
                v(0), kq(0, 1, 0), v(1), v(2), v(3), kq(0, 2, 0),
                v(4), v(5), v(6), kq(0, 3, 0), v(7), v(8), v(9),
                kq(0, 1, 1), v(10), v(11), v(12), v(13), v(14), v(15),
                kq(1, 0, 0), kq(1, 0, 1),
            ]
            run_block(r, 0, 0, extrasA)

            extrasB = [
                kq(1, 1, 0), kq(1, 2, 0), kq(1, 3, 0), kq(1, 1, 1),
                kq(0, 2, 1), kq(0, 3, 1),
            ]
            run_block(r, 0, 1, extrasB, lag=4)

            extrasC = [
                fin_piece(r, 0), None, fin_piece(r, 1), None,
                fin_piece(r, 2), None, fin_piece(r, 3), None,
                fin_piece(r, 4), None, fin_piece(r, 5), None,
                fin_piece(r, 6), None, fin_piece(r, 7), None,
                kq(1, 2, 1), None, kq(1, 3, 1),
            ]
            run_block(r, 1, 0, extrasC, lag=4)

            if r + 1 < repeat:
                dmas(r + 1)

            extrasD = [None] * 21 + [
                fin_piece(r, 8), None, fin_piece(r, 9), None,
                fin_piece(r, 10), None, fin_piece(r, 11),
            ]
            run_block(r, 1, 1, extrasD, lag=3)
            drain_pv(r)

            if r + 1 < repeat:
                kq_piece(r + 1, 0, 0, 0)()
                kq_piece(r + 1, 0, 0, 1)()
            for q8 in range(12, 16):
                fin_piece(r, q8, pool="o")()

    nc.compile()
    return nc


# ---------------- host-side helpers ----------------


def rel_pos_enc(seq_len, dim):
    positions = np.arange(seq_len, dtype=np.float32)[:, None]
    div_term = np.exp(
        np.arange(0, dim, 2, dtype=np.float32) * (-(np.log(10000.0) / dim))
    )
    pe = np.zeros((seq_len, dim), dtype=np.float32)
    pe[:, 0::2] = np.sin(positions * div_term)
    pe[:, 1::2] = np.cos(positions * div_term)
    return pe


def core_inputs(x, W_qkv, b_qkv, core):
    b = core // 4
    h0 = (core % 4) * 4
    cols = slice(h0 * 64, (h0 + 4) * 64)
    xt = np.ascontiguousarray(x[b].T)
    wq = np.ascontiguousarray(W_qkv[:, 0:1024][:, cols])
    wk = np.ascontiguousarray(W_qkv[:, 1024:2048][:, cols])
    wv = np.ascontiguousarray(W_qkv[:, 2048:3072][:, cols])
    bq = b_qkv[0:1024][cols]
    bk = b_qkv[1024:2048][cols]
    bqk = np.stack(
        [bq[0:128], bq[128:256], bk[0:128], bk[128:256]], axis=1
    ).astype(np.float32)
    pos = rel_pos_enc(S, HD)  # [S, 64]
    post = np.ascontiguousarray(
        np.concatenate([pos.T, pos.T], axis=0).astype(np.float32)
    )  # [128, S]
    return {
        "xt": xt,
        "wq": wq,
        "wk": wk,
        "wv": wv,
        "bqk": np.ascontiguousarray(bqk),
        "post": post,
    }


def core_inputs_out(W_out, core):
    h0 = (core % 4) * 4
    rows = slice(h0 * 64, (h0 + 4) * 64)
    return {"wo": np.ascontiguousarray(W_out[rows, :])}


def all_core_inputs(x, W_qkv, b_qkv, W_out):
    ins = []
    for c in range(8):
        m = core_inputs(x, W_qkv, b_qkv, c)
        m.update(core_inputs_out(W_out, c))
        ins.append(m)
    return ins


def combine_outputs(partials, b_qkv, W_out, b_out):
    extra = b_qkv[2048:3072] @ W_out + b_out  # [DM]
    outs = []
    for b in range(2):
        acc = partials[b * 4].astype(np.float64)
        for c in range(b * 4 + 1, b * 4 + 4):
            acc = acc + partials[c]
        outs.append((acc + extra).astype(np.float32))
    return np.stack(outs, axis=0)  # [2, S, DM]


_CACHE = {}


def _get_program():
    if "nc" not in _CACHE:
        _CACHE["nc"] = build_program(use_f32r=True)
    return _CACHE["nc"]


def kernel(x, W_qkv, b_qkv, W_out, b_out):
    x = np.ascontiguousarray(np.asarray(x, dtype=np.float32))
    W_qkv = np.ascontiguousarray(np.asarray(W_qkv, dtype=np.float32))
    b_qkv = np.asarray(b_qkv, dtype=np.float32)
    W_out = np.ascontiguousarray(np.asarray(W_out, dtype=np.float32))
    b_out = np.asarray(b_out, dtype=np.float32)

    from concourse import bass_utils

    nc = _get_program()
    in_maps = all_core_inputs(x, W_qkv, b_qkv, W_out)
    res = bass_utils.run_bass_kernel_spmd(nc, in_maps, core_ids=list(range(8)))
    partials = [res.results[c]["out"] for c in range(8)]
    return combine_outputs(partials, b_qkv, W_out, b_out)
